# revision 9
# baseline (speedup 1.0000x reference)
"""Trainium2 Bass kernel for nn_NN_split_14516989460965 (segment_reduce).

Reference computation (B=1024, N_GENES=20000, H=2048, G=64, OUT=1):
    gathered = x[:, group_idx]                        # [B, H, G]
    hidden = einsum('bhg,hg->bh', gathered, group_w) + group_b
    out = sigmoid(relu(hidden) @ w2.T + b2)           # [B, 1]

Strategy
--------
The gather+segment-reduce is a sparse matmul: hidden = x @ W_s where
W_s[n, h] = sum_g group_w[h, g] * [group_idx[h, g] == n].  Fine-grained
gather is hopeless on TRN2 (no DVE gather; GPSIMD random SBUF reads are
~102 cyc each), so densify W_s on the host (it is built from the tiny
idx/w tensors) and run a dense bf16 GEMM on the PE array.

Sharding: split the contraction (gene) dimension across the 8 cores —
each core gets x[:, 2500c:2500(c+1)] transposed to [2500, 1024] bf16
plus its [2500, 2048] slice of the dense weights (bf16).  Each core
computes a partial hidden[1024, 2048]; a ReduceScatter sums partials
and hands core c batch rows [128c, 128(c+1)).  Groups are processed in
two halves so the first half's ReduceScatter overlaps the second
half's GEMM.

fc2 folding: |w2| is folded into the dense weight columns
(relu(h)*w2 = sign(w2)*relu(h*|w2|)) and groups are permuted so
positive-sign groups come first.  group_b (scaled by |w2|) rides as an
extra GEMM row (all-ones x row) on core 0 only.  The on-device tail is
then just: relu+accumulate over the positive slice, same over the
negative slice, logit = pos - neg, sigmoid(logit + b2).
"""

import numpy as np
import ml_dtypes

import concourse.bass as bass
import concourse.bacc as bacc
import concourse.mybir as mybir
import concourse.tile as tile
from concourse.bass_utils import run_bass_kernel_spmd

# Problem shapes (hardcoded per contract)
B = 1024
N_GENES = 20000
H = 2048
G = 64
NCORES = 8
K_LOC = N_GENES // NCORES          # 2500 genes per core
KT = 20                            # k-tiles of 128 per core
K_PAD = KT * 128                   # 2560 (padding rows zero; row K_LOC = bias row)
BT = B // 128                      # 8 batch tiles
# Uneven group chunks: big first chunks overlap their ReduceScatter with the
# remaining GEMM; a small last chunk minimizes the un-overlapped RS tail.
CHUNKS = (1024, 640, 384)
CHUNK_OFF = (0, 1024, 1664)
MM_N = 512                         # max moving free dim per matmul

BF16 = mybir.dt.bfloat16
F32 = mybir.dt.float32

_NC_CACHE = {}


def _build_bass(neg_start: int, b2_val: float):
    nc = bacc.Bacc("TRN2", target_bir_lowering=False, debug=False, num_devices=NCORES)

    xt = nc.declare_dram_parameter("xt", [K_PAD, B], BF16, isOutput=False)
    w = nc.declare_dram_parameter("w", [K_PAD, H], BF16, isOutput=False)
    out = nc.declare_dram_parameter("out", [128, 1], F32, isOutput=True)

    partials = [
        nc.dram_tensor(f"partial{h}", [B, hh], BF16) for h, hh in enumerate(CHUNKS)
    ]
    reduceds = [
        nc.dram_tensor(f"reduced{h}", [128, hh], BF16) for h, hh in enumerate(CHUNKS)
    ]

    with tile.TileContext(nc) as tc:
        with (
            tc.tile_pool(name="resident", bufs=1) as res_pool,
            tc.tile_pool(name="psum", bufs=4, space="PSUM") as psum_pool,
            tc.tile_pool(name="dump", bufs=3) as dump_pool,
            tc.tile_pool(name="tail", bufs=1) as tail_pool,
        ):
            # Resident operands (bf16 on the wire).
            xt_bf = res_pool.tile([128, KT * B], BF16, tag="xt_bf")
            w_bf = res_pool.tile([128, KT * H], BF16, tag="w_bf")
            for k in range(KT):
                nc.sync.dma_start(
                    xt_bf[:, k * B:(k + 1) * B], xt[k * 128:(k + 1) * 128, :]
                )
                nc.sync.dma_start(
                    w_bf[:, k * H:(k + 1) * H], w[k * 128:(k + 1) * 128, :]
                )

            # Dense GEMM chunk by chunk; RS of chunk h overlaps later chunks.
            pos_accs = []
            neg_accs = []
            for nh, hh in enumerate(CHUNKS):
                off = CHUNK_OFF[nh]
                nsplits = [(i, min(MM_N, hh - i)) for i in range(0, hh, MM_N)]
                for bt in range(BT):
                    psum = psum_pool.tile(
                        [128, hh], F32, tag="ps", name=f"ps_{nh}_{bt}",
                        padded_shape=[128, 1024],
                    )
                    for k in range(KT):
                        lhsT = xt_bf[:, k * B + bt * 128: k * B + (bt + 1) * 128]
                        for n0, nw in nsplits:
                            nc.tensor.matmul(
                                psum[:, n0:n0 + nw],
                                lhsT,
                                w_bf[:, k * H + off + n0: k * H + off + n0 + nw],
                                start=(k == 0),
                                stop=(k == KT - 1),
                            )
                    pt = dump_pool.tile(
                        [128, hh], BF16, tag="pdump", name=f"pd_{nh}_{bt}",
                        padded_shape=[128, 1024],
                    )
                    for n0, nw in nsplits:
                        nc.scalar.copy(pt[:, n0:n0 + nw], psum[:, n0:n0 + nw])
                    nc.sync.dma_start(
                        partials[nh][bt * 128:(bt + 1) * 128, :], pt[:]
                    )
                nc.gpsimd.collective_compute(
                    "ReduceScatter",
                    mybir.AluOpType.add,
                    replica_groups=[list(range(NCORES))],
                    ins=[partials[nh][:, :]],
                    outs=[reduceds[nh][:, :]],
                )
                # Per-chunk tail: relu + free-axis accumulate per sign bucket.
                hred = tail_pool.tile(
                    [128, hh], BF16, tag=f"hred{nh}", name=f"hred_{nh}",
                )
                nc.sync.dma_start(hred[:], reduceds[nh][:, :])
                scratch = tail_pool.tile(
                    [128, hh], BF16, tag=f"scratch{nh}", name=f"scratch_{nh}",
                )
                # columns [off, off+hh) of the permuted order; split at neg_start
                p_hi = max(0, min(hh, neg_start - off))   # [0, p_hi) positive
                if p_hi > 0:
                    acc = tail_pool.tile([128, 1], F32, name=f"accp_{nh}")
                    nc.scalar.activation(
                        scratch[:, :p_hi],
                        hred[:, :p_hi],
                        mybir.ActivationFunctionType.Relu,
                        accum_out=acc[:],
                    )
                    pos_accs.append(acc)
                if p_hi < hh:
                    acc = tail_pool.tile([128, 1], F32, name=f"accn_{nh}")
                    nc.scalar.activation(
                        scratch[:, p_hi:],
                        hred[:, p_hi:],
                        mybir.ActivationFunctionType.Relu,
                        accum_out=acc[:],
                    )
                    neg_accs.append(acc)

            logit = tail_pool.tile([128, 1], F32, tag="logit")
            psum_acc = tail_pool.tile([128, 1], F32, tag="psum_acc")
            nsum_acc = tail_pool.tile([128, 1], F32, tag="nsum_acc")
            for accs, dst in ((pos_accs, psum_acc), (neg_accs, nsum_acc)):
                nc.vector.tensor_copy(dst[:], accs[0][:])
                for a in accs[1:]:
                    nc.vector.tensor_add(dst[:], dst[:], a[:])
            nc.vector.tensor_sub(logit[:], psum_acc[:], nsum_acc[:])
            b2_t = tail_pool.tile([128, 1], F32, tag="b2c")
            nc.vector.memset(b2_t[:], float(b2_val))
            res = tail_pool.tile([128, 1], F32, tag="res")
            nc.scalar.activation(
                res[:],
                logit[:],
                mybir.ActivationFunctionType.Sigmoid,
                bias=b2_t[:],
            )
            nc.sync.dma_start(out[:, :], res[:])

    nc.compile()
    return nc


def _prep_inputs(x, group_idx, group_w, group_b, w2):
    """Densify + permute + scale weights; shard genes across cores."""
    w2_flat = w2.reshape(-1).astype(np.float64)
    pos = np.nonzero(w2_flat >= 0)[0]
    neg = np.nonzero(w2_flat < 0)[0]
    perm = np.concatenate([pos, neg])          # column position -> group
    neg_start = int(len(pos))
    col_of_group = np.empty(H, dtype=np.int64)
    col_of_group[perm] = np.arange(H)

    scale = np.abs(w2_flat)                    # per original group h

    # Dense W_s[n, col] = sum_g w[h,g]*|w2[h]| over idx[h,g]==n, col=col_of_group[h]
    wd = np.zeros((N_GENES, H), dtype=np.float32)
    rows = group_idx.reshape(-1).astype(np.int64)
    cols = np.repeat(col_of_group, G)
    vals = (group_w.astype(np.float64) * scale[:, None]).reshape(-1).astype(np.float32)
    np.add.at(wd, (rows, cols), vals)

    b_scaled = np.zeros(H, dtype=np.float32)
    b_scaled[col_of_group] = (group_b.astype(np.float64) * scale).astype(np.float32)

    xt_shards = []
    w_shards = []
    x_bf = x.astype(ml_dtypes.bfloat16)
    for c in range(NCORES):
        sl = slice(c * K_LOC, (c + 1) * K_LOC)
        xt_c = np.zeros((K_PAD, B), dtype=ml_dtypes.bfloat16)
        xt_c[:K_LOC, :] = x_bf[:, sl].T
        xt_c[K_LOC, :] = 1.0                   # bias row (ones)
        w_c = np.zeros((K_PAD, H), dtype=np.float32)
        w_c[:K_LOC, :] = wd[sl, :]
        if c == 0:
            w_c[K_LOC, :] = b_scaled           # bias only on core 0
        xt_shards.append(np.ascontiguousarray(xt_c))
        w_shards.append(w_c.astype(ml_dtypes.bfloat16))
    return xt_shards, w_shards, neg_start


def kernel(x, group_idx, group_w, group_b, w2, b2, _profile=False):
    xt_shards, w_shards, neg_start = _prep_inputs(x, group_idx, group_w, group_b, w2)
    b2_val = float(np.asarray(b2).reshape(-1)[0])

    key = (neg_start, round(b2_val, 9))
    nc = _NC_CACHE.get(key)
    if nc is None:
        nc = _build_bass(neg_start, b2_val)
        _NC_CACHE[key] = nc

    in_maps = [{"xt": xt_shards[c], "w": w_shards[c]} for c in range(NCORES)]
    res = run_bass_kernel_spmd(
        nc, in_maps, core_ids=list(range(NCORES)), trace=_profile
    )
    out = np.concatenate(
        [res.results[c]["out"] for c in range(NCORES)], axis=0
    ).astype(np.float32)
    if _profile:
        return out, res
    return out
